# revision 11
# baseline (speedup 1.0000x reference)
"""Trainium2 Bass kernel for nn_DataEmbedding_Stats.

Computation: rolling-window stats (window=24, replicate-padded) over
x (B,S,7) -> 35 features -> circular conv1d(k=3) -> (B,S,512).

Strategy (8 NeuronCores, pure data parallel over batch, 4 batches/core):
 - x loaded as 32 [128,28] staging DMAs + PE transposes into
   X [128, 1048]: partition = 32j + 7b + c (j = 1024-seq chunk), col 0 = 0,
   cols 1..23 = replicate halo, col 24+s = x at in-chunk seq s.
 - window sums via cumsum (tensor_tensor_scan) + shift-24 subtract; var from
   sum/sumsq; max/min via log-doubling ladders (levels 2+ in bf16).
 - stats finals write bf16 directly into ST2ALL [28, 5*4100]: partition
   7b+c, stat t block at cols 4100t, block col 2+s (+2-col wraps for the
   circular conv).
 - F3 [106, 4098] per batch (im2col for the k=3 conv) built with 3 DMAs
   (one per tap k): dst rows 35k + 5c + t, src = ST2ALL[7b+c, 4100t+k+m]
   via a custom 3-dim access pattern. Weight rows reordered on host to
   match; row 105 = ones (bias folded); mean = S24 * (W/24 fold).
 - conv as matmul: per 128 positions, psum[128,512] = F3[:,p+1:p+129].T @ wt.
 - PSUM -> SBUF copies downcast to bf16 (DVE/ACT alternating); bf16 output
   staged [128, 4096] and stored as 1 MB DMAs on both HWDGE queues; host
   upcasts to f32.
"""

import dataclasses

import numpy as np

try:
    import concourse.bass as bass  # noqa: F401
except ImportError:
    import sys

    for _p in ("/opt/trn_rl_repo", "/root/.axon_site/_ro/trn_rl_repo"):
        if _p not in sys.path:
            sys.path.insert(0, _p)

B, S, C, W, D = 32, 4096, 7, 24, 512
NCORES = 8
BSH = B // NCORES          # batches per core
NJ = 4                     # seq chunks (row groups of 32 partitions)
CH = S // NJ               # 1024
HALO = W - 1               # 23
XC = 1 + HALO + CH         # 1048: col 0 = 0, 1..23 halo, 24+s data
NF = 5 * C                 # 35 features
K = 3 * NF + 1             # 106 contraction rows (ones row last)
F3W = S + 2                # 4098
SBW = S + 4                # 4100: per-stat block width, block col = 2+s
STW = 5 * SBW              # 20500: ST2ALL width (5 stat blocks)
NT = S // 128              # 32 position tiles per batch
OUTG = 8                   # position tiles per output staging tile

_CACHE = {}


def _build():
    import concourse.bacc as bacc
    import concourse.tile as tile
    from concourse import mybir

    f32 = mybir.dt.float32
    bf16 = mybir.dt.bfloat16
    Alu = mybir.AluOpType
    Act = mybir.ActivationFunctionType

    nc = bacc.Bacc(
        "TRN2",
        target_bir_lowering=False,
        debug=False,
        enable_asserts=False,
        num_devices=NCORES,
    )

    x_d = nc.dram_tensor("x", (BSH, S, C), f32, kind="ExternalInput")
    wt_d = nc.dram_tensor("wt", (K, D), bf16, kind="ExternalInput")
    ones_d = nc.dram_tensor("ones", (1, F3W), bf16, kind="ExternalInput")
    id_d = nc.dram_tensor("ident", (128, 128), f32, kind="ExternalInput")
    y_d = nc.dram_tensor("y", (BSH, S, D), bf16, kind="ExternalOutput")

    with tile.TileContext(nc) as tc:
        with (
            tc.tile_pool(name="stats", bufs=1) as pst,
            tc.tile_pool(name="st2p", bufs=1) as pst2,
            tc.tile_pool(name="f3p", bufs=4) as pf3,
            tc.tile_pool(name="wtp", bufs=1) as pwt,
            tc.tile_pool(name="stage_in", bufs=12) as pstg,
            tc.tile_pool(name="psT", bufs=2, space="PSUM") as psT,
            tc.tile_pool(name="psum", bufs=3, space="PSUM") as pps,
            tc.tile_pool(name="outp", bufs=4) as pout,
        ):
            wt = pwt.tile([K, D], bf16, tag="wt")
            nc.sync.dma_start(wt[:], wt_d.ap())
            ident = pwt.tile([128, 128], f32, tag="ident")
            nc.sync.dma_start(ident[:], id_d.ap())

            X = pst.tile([128, XC], f32, tag="X")
            SQX = pst.tile([128, XC], f32, tag="SQX")
            P = pst.tile([128, XC], f32, tag="P")
            Q = pst.tile([128, XC], f32, tag="Q")
            S24 = pst.tile([128, XC], f32, tag="S24")
            SQ24 = pst.tile([128, XC], f32, tag="SQ24")
            M2 = pst.tile([128, XC], f32, tag="M2")
            XB = pst.tile([128, XC], bf16, tag="XB")
            B1 = pst.tile([128, XC], bf16, tag="B1")
            B2 = pst.tile([128, XC], bf16, tag="B2")
            B3 = pst.tile([128, XC], bf16, tag="B3")
            B4 = pst.tile([128, XC], bf16, tag="B4")

            ST2 = pst2.tile([28, STW], bf16, tag="ST2ALL")

            nc.vector.memset(X[:, 0:1], 0.0)
            # preload ACT function tables (Square/Sqrt) off the critical path
            nc.scalar.square(SQX[0:1, 0:2], ident[0:1, 0:2])
            nc.scalar.activation(M2[0:1, 0:2], ident[0:1, 0:2], Act.Sqrt, 0.0, 1.0)

            # ---- load x: one staging tile [128, 896], col = 28*(s//128) +
            # 7b + c, partition = s % 128. One DMA per batch (strided dst,
            # 3-dim AP), then 8 PE transposes of [128, 112] (one per 512-seq
            # block T) -> psum [112, 128] (partition 28u + 7b + c), then 4
            # copies per T into X rows 32j+7b+c at col 24 + 512(T%2) + 128u.
            stg = pstg.tile([128, 1024], f32, tag="stg")
            for b in range(BSH):
                dst = dataclasses.replace(
                    stg[:], ap=[[1024, 128], [32, 32], [1, 7]], offset=7 * b
                )
                src = dataclasses.replace(
                    x_d.ap(),
                    ap=[[C, 128], [128 * C, 32], [1, C]],
                    offset=b * S * C,
                )
                eng = (nc.sync, nc.scalar, nc.gpsimd, nc.sync)[b]
                eng.dma_start(dst, src)
            nblk = 0
            for T in range(8):
                j = T // 2
                pst_t = psT.tile([128, 128], f32, tag="pst_t")
                nc.tensor.transpose(
                    pst_t[:], stg[:, 128 * T : 128 * (T + 1)], ident[:]
                )
                for u in range(4):
                    c0 = 24 + 512 * (T % 2) + 128 * u
                    ceng = (nc.scalar.copy, nc.vector.tensor_copy)[nblk % 2]
                    ceng(
                        X[32 * j : 32 * j + 28, c0 : c0 + 128],
                        pst_t[32 * u : 32 * u + 28, :],
                    )
                    nblk += 1
                if T % 2 == 1 and j + 1 < NJ:
                    # back-halo for chunk j+1: last 23 seq of chunk j
                    nc.vector.tensor_copy(
                        X[32 * (j + 1) : 32 * (j + 1) + 28, 1 : 1 + HALO],
                        pst_t[96 : 96 + 28, 128 - HALO : 128],
                    )
                if T == 0:
                    # j=0 front halo: replicate x[b,0,c] into cols 1..23
                    nc.vector.tensor_scalar(
                        X[0:28, 1 : 1 + HALO],
                        X[0:28, 24 : 24 + HALO],
                        0.0,
                        X[0:28, 24:25],
                        Alu.mult,
                        Alu.add,
                    )

            # ---- stats
            # window sums: P = cumsum(X), S24[m] = P[m] - P[m-24] (m >= 24;
            # col 0 of X is zero so the first window is exact).
            nc.scalar.square(SQX[:, 0:XC], X[:, 0:XC])
            nc.vector.tensor_tensor_scan(
                P[:, 0:XC], X[:, 0:XC], X[:, 0:XC], 0.0, Alu.add, Alu.bypass
            )
            nc.vector.tensor_tensor_scan(
                Q[:, 0:XC], SQX[:, 0:XC], SQX[:, 0:XC], 0.0, Alu.add, Alu.bypass
            )
            nc.vector.tensor_tensor(
                S24[:, 24:XC], P[:, 24:XC], P[:, 0:CH], Alu.subtract
            )
            nc.vector.tensor_tensor(
                SQ24[:, 24:XC], Q[:, 24:XC], Q[:, 0:CH], Alu.subtract
            )
            # var = max(SQ24 - S24^2/24, 0) / 23 ; std = sqrt(var)
            nc.scalar.activation(
                M2[:, 24:XC], S24[:, 24:XC], Act.Square, 0.0, float(W**-0.5)
            )
            nc.vector.tensor_tensor(
                SQ24[:, 24:XC], SQ24[:, 24:XC], M2[:, 24:XC], Alu.subtract
            )
            nc.vector.tensor_scalar(
                SQ24[:, 24:XC], SQ24[:, 24:XC], 0.0, None, Alu.max
            )

            # max/min ladders: w2/w4/w8/w16, then w24[m]=op(w16[m], w8[m-16])
            nc.scalar.copy(XB[:, 1:XC], X[:, 1:XC])
            nc.vector.tensor_tensor(
                B1[:, 2:XC], XB[:, 2:XC], XB[:, 1 : XC - 1], Alu.max
            )
            nc.vector.tensor_tensor(
                B3[:, 2:XC], XB[:, 2:XC], XB[:, 1 : XC - 1], Alu.min
            )
            nc.vector.tensor_tensor(
                B2[:, 4:XC], B1[:, 4:XC], B1[:, 2 : XC - 2], Alu.max
            )
            nc.vector.tensor_tensor(
                B4[:, 4:XC], B3[:, 4:XC], B3[:, 2 : XC - 2], Alu.min
            )
            nc.vector.tensor_tensor(
                B1[:, 8:XC], B2[:, 8:XC], B2[:, 4 : XC - 4], Alu.max
            )
            nc.vector.tensor_tensor(
                B3[:, 8:XC], B4[:, 8:XC], B4[:, 4 : XC - 4], Alu.min
            )
            nc.vector.tensor_tensor(
                B2[:, 16:XC], B1[:, 16:XC], B1[:, 8 : XC - 8], Alu.max
            )
            nc.vector.tensor_tensor(
                B4[:, 16:XC], B3[:, 16:XC], B3[:, 8 : XC - 8], Alu.min
            )

            # ---- finals write straight into ST2ALL (stat t at cols 4100t,
            # block col = 2 + seq). Order: x, mean(=S24), max, min, std.
            # F3 row 35k + 5c + t (tap k, channel c, stat t), row 105 = ones.
            # F3[r, m] = ST2[7b+c, 4100t + k + m]. Each tap is built in
            # chunk-aligned pieces so matmuls start after chunks 0-1:
            #   A: m in [2-k, 2050-k)   <- chunks 0,1
            #   B: m in [2050-k, 3074-k) <- chunk 2
            #   C: m in [3074-k, 4098)  <- chunk 3 + right wrap
            #   edge: m in [0, 2-k)     <- left wrap (k < 2)
            f3s = [
                pf3.tile([K, F3W], bf16, tag="F3", name=f"f3_{b}")
                for b in range(BSH)
            ]
            for b in range(BSH):
                nc.sync.dma_start(f3s[b][K - 1 : K, :], ones_d.ap())

            def sc(t, j):
                return SBW * t + 2 + CH * j

            def finals(j):
                r0, r1 = 32 * j, 32 * j + 28
                nc.scalar.copy(ST2[:, sc(0, j) : sc(0, j) + CH], X[r0:r1, 24:XC])
                nc.vector.tensor_tensor(
                    ST2[:, sc(1, j) : sc(1, j) + CH],
                    P[r0:r1, 24:XC],
                    P[r0:r1, 0:CH],
                    Alu.subtract,
                )
                nc.vector.tensor_tensor(
                    ST2[:, sc(2, j) : sc(2, j) + CH],
                    B2[r0:r1, 24:XC],
                    B1[r0:r1, 8 : 8 + CH],
                    Alu.max,
                )
                nc.vector.tensor_tensor(
                    ST2[:, sc(3, j) : sc(3, j) + CH],
                    B4[r0:r1, 24:XC],
                    B3[r0:r1, 8 : 8 + CH],
                    Alu.min,
                )
                nc.scalar.activation(
                    ST2[:, sc(4, j) : sc(4, j) + CH],
                    SQ24[r0:r1, 24:XC],
                    Act.Sqrt,
                    0.0,
                    1.0 / (W - 1),
                )

            def wrapcopy(dst_off, src_off):
                dst = dataclasses.replace(
                    ST2[:, 0:2], ap=[[STW, 28], [SBW, 5], [1, 2]], offset=dst_off
                )
                src = dataclasses.replace(
                    ST2[:, 0:2], ap=[[STW, 28], [SBW, 5], [1, 2]], offset=src_off
                )
                nc.vector.tensor_copy(dst, src)

            nf3 = 0

            def f3dma(b, k, m0, m1, eng=None):
                nonlocal nf3
                src = dataclasses.replace(
                    ST2[:],
                    ap=[[STW, 7], [SBW, 5], [1, m1 - m0]],
                    offset=7 * b * STW + k + m0,
                )
                if eng is None:
                    eng = (nc.sync, nc.scalar)[nf3 % 2]
                    nf3 += 1
                eng.dma_start(f3s[b][35 * k : 35 * k + 35, m0:m1], src)

            finals(0)
            finals(1)
            wrapcopy(2 + S, 2)  # right wrap <- seq 0:2 (chunk 0)
            for b in range(BSH):
                for k in range(3):
                    f3dma(b, k, 2 - k, 2050 - k)  # A
            finals(2)
            for b in range(BSH):
                for k in range(3):
                    f3dma(b, k, 2050 - k, 3074 - k, eng=nc.gpsimd)  # B
            finals(3)
            wrapcopy(0, 2 + S - 2)  # left wrap <- seq 4094:4096 (chunk 3)
            for b in range(BSH):
                for k in range(3):
                    f3dma(b, k, 3074 - k, F3W)  # C
                for k in range(2):
                    f3dma(b, k, 0, 2 - k, eng=nc.gpsimd)  # edge (left wraps)

            # ---- matmuls, group-major so early groups span all batches.
            # Group g < 3: tiles 1+8g .. 8+8g -> y rows 128+1024g ..;
            # last group: tiles 25..31 and 0 (needs the edge cols).
            GROUPS = [list(range(1 + 8 * g, 9 + 8 * g)) for g in range(3)]
            GROUPS.append([25, 26, 27, 28, 29, 30, 31, 0])
            ncopy = 0
            for gi, tiles in enumerate(GROUPS):
                for b in range(BSH):
                    stage = pout.tile([128, OUTG * D], bf16, tag="stage")
                    for pi in range(4):
                        ps = pps.tile([128, 2 * D], f32, tag="ps")
                        for h in range(2):
                            q = tiles[2 * pi + h]
                            nc.tensor.matmul(
                                ps[:, D * h : D * (h + 1)],
                                f3s[b][:, 128 * q + 1 : 128 * q + 129],
                                wt[:],
                                start=True,
                                stop=True,
                            )
                        ceng = (
                            nc.vector.tensor_copy
                            if ncopy % 8 in (1, 4, 6)
                            else nc.scalar.copy
                        )
                        ceng(stage[:, 1024 * pi : 1024 * (pi + 1)], ps[:])
                        ncopy += 1
                    deng = nc.sync if (gi * 4 + b) % 2 == 0 else nc.scalar
                    if gi < 3:
                        deng.dma_start(
                            y_d.ap()[
                                b, 128 + 1024 * gi : 128 + 1024 * (gi + 1), :
                            ].rearrange("(q p) d -> p q d", p=128),
                            stage[:].rearrange("p (q d) -> p q d", q=OUTG),
                        )
                    else:
                        deng.dma_start(
                            y_d.ap()[b, 3200:S, :].rearrange(
                                "(q p) d -> p q d", p=128
                            ),
                            stage[:, 0 : 7 * D].rearrange(
                                "p (q d) -> p q d", q=7
                            ),
                        )
                        deng.dma_start(
                            y_d.ap()[b, 0:128, :], stage[:, 7 * D : 8 * D]
                        )

    nc.compile()
    return nc


def _prep_host(W_conv, b_conv):
    import ml_dtypes

    # wt row = 35k + 5c + t; conv in-channel = 7t + c; mean fold 1/24.
    wkf = np.ascontiguousarray(W_conv.transpose(2, 1, 0)).copy()  # (3, 35, 512)
    wkf[:, C : 2 * C, :] *= 1.0 / W
    wt = np.empty((K, D), np.float32)
    kk, tt, cc = np.meshgrid(
        np.arange(3), np.arange(5), np.arange(C), indexing="ij"
    )
    wt[(35 * kk + 5 * cc + tt).ravel()] = wkf.reshape(3, 35, D)[
        kk.ravel(), (7 * tt + cc).ravel()
    ]
    wt[K - 1] = b_conv.astype(np.float32)
    return wt.astype(ml_dtypes.bfloat16)


def _run(x, W_conv, b_conv, trace=False, **kw):
    from concourse import bass_utils

    if "nc" not in _CACHE:
        _CACHE["nc"] = _build()
    nc = _CACHE["nc"]

    wt = _prep_host(np.asarray(W_conv), np.asarray(b_conv))
    import ml_dtypes

    ones = np.ones((1, F3W), ml_dtypes.bfloat16)
    ident = np.eye(128, dtype=np.float32)
    x = np.ascontiguousarray(np.asarray(x, np.float32))
    in_maps = [
        {"x": x[BSH * i : BSH * (i + 1)], "wt": wt, "ones": ones, "ident": ident}
        for i in range(NCORES)
    ]
    res = bass_utils.run_bass_kernel_spmd(
        nc, in_maps, core_ids=list(range(NCORES)), trace=trace, **kw
    )
    out = np.concatenate(
        [np.asarray(r["y"]).astype(np.float32) for r in res.results], axis=0
    )
    return out, res


def kernel(x, x_mark=None, W_conv=None, b_conv=None, **_unused):
    out, _ = _run(x, W_conv, b_conv, trace=False)
    return out


# revision 15
# speedup vs baseline: 1.1450x; 1.1450x over previous
"""Trainium2 Bass kernel for nn_DataEmbedding_Stats.

Computation: rolling-window stats (window=24, replicate-padded) over
x (B,S,7) -> 35 features -> circular conv1d(k=3) -> (B,S,512).

Strategy (8 NeuronCores, pure data parallel over batch, 4 batches/core):
 - x loaded as 32 [128,28] staging DMAs + PE transposes into
   X [128, 1048]: partition = 32j + 7b + c (j = 1024-seq chunk), col 0 = 0,
   cols 1..23 = replicate halo, col 24+s = x at in-chunk seq s.
 - window sums via cumsum (tensor_tensor_scan) + shift-24 subtract; var from
   sum/sumsq; max/min via log-doubling ladders (levels 2+ in bf16).
 - stats finals write bf16 directly into ST2ALL [28, 5*4100]: partition
   7b+c, stat t block at cols 4100t, block col 2+s (+2-col wraps for the
   circular conv).
 - F3 [106, 4098] per batch (im2col for the k=3 conv) built with 3 DMAs
   (one per tap k): dst rows 35k + 5c + t, src = ST2ALL[7b+c, 4100t+k+m]
   via a custom 3-dim access pattern. Weight rows reordered on host to
   match; row 105 = ones (bias folded); mean = S24 * (W/24 fold).
 - conv as matmul: per 128 positions, psum[128,512] = F3[:,p+1:p+129].T @ wt.
 - PSUM -> SBUF copies downcast to bf16 (DVE/ACT alternating); bf16 output
   staged [128, 4096] and stored as 1 MB DMAs on both HWDGE queues; host
   upcasts to f32.
"""

import dataclasses

import numpy as np

try:
    import concourse.bass as bass  # noqa: F401
except ImportError:
    import sys

    for _p in ("/opt/trn_rl_repo", "/root/.axon_site/_ro/trn_rl_repo"):
        if _p not in sys.path:
            sys.path.insert(0, _p)

B, S, C, W, D = 32, 4096, 7, 24, 512
NCORES = 8
BSH = B // NCORES          # batches per core
NJ = 4                     # seq chunks (row groups of 32 partitions)
CH = S // NJ               # 1024
HALO = W - 1               # 23
XC = 1 + HALO + CH         # 1048: col 0 = 0, 1..23 halo, 24+s data
NF = 5 * C                 # 35 features
K = 3 * NF + 1             # 106 contraction rows (ones row last)
F3W = S + 2                # 4098
SBW = S + 4                # 4100: per-stat block width, block col = 2+s
STW = 5 * SBW              # 20500: ST2ALL width (5 stat blocks)
NT = S // 128              # 32 position tiles per batch
OUTG = 8                   # position tiles per output staging tile

_CACHE = {}


def _build():
    import concourse.bacc as bacc
    import concourse.tile as tile
    from concourse import mybir

    f32 = mybir.dt.float32
    bf16 = mybir.dt.bfloat16
    Alu = mybir.AluOpType
    Act = mybir.ActivationFunctionType

    nc = bacc.Bacc(
        "TRN2",
        target_bir_lowering=False,
        debug=False,
        enable_asserts=False,
        num_devices=NCORES,
    )

    x_d = nc.dram_tensor("x", (BSH, S, C), f32, kind="ExternalInput")
    wt_d = nc.dram_tensor("wt", (K, D), bf16, kind="ExternalInput")
    ones_d = nc.dram_tensor("ones", (1, F3W), bf16, kind="ExternalInput")
    id_d = nc.dram_tensor("ident", (128, 128), f32, kind="ExternalInput")
    y_d = nc.dram_tensor("y", (BSH, S, D), bf16, kind="ExternalOutput")

    with tile.TileContext(nc) as tc:
        with (
            tc.tile_pool(name="stats", bufs=1) as pst,
            tc.tile_pool(name="st2p", bufs=1) as pst2,
            tc.tile_pool(name="f3p", bufs=4) as pf3,
            tc.tile_pool(name="wtp", bufs=1) as pwt,
            tc.tile_pool(name="stage_in", bufs=12) as pstg,
            tc.tile_pool(name="psum", bufs=4, space="PSUM") as pps,
            tc.tile_pool(name="outp", bufs=4) as pout,
        ):
            wt = pwt.tile([K, D], bf16, tag="wt")
            nc.sync.dma_start(wt[:], wt_d.ap())
            ident = pwt.tile([128, 128], f32, tag="ident")
            nc.sync.dma_start(ident[:], id_d.ap())

            X = pst.tile([128, XC], f32, tag="X")
            SQX = pst.tile([128, XC], f32, tag="SQX")
            P = pst.tile([128, XC], f32, tag="P")
            Q = pst.tile([128, XC], f32, tag="Q")
            S24 = pst.tile([128, XC], f32, tag="S24")
            SQ24 = pst.tile([128, XC], f32, tag="SQ24")
            M2 = pst.tile([128, XC], f32, tag="M2")
            XB = pst.tile([128, XC], bf16, tag="XB")
            B1 = pst.tile([128, XC], bf16, tag="B1")
            B2 = pst.tile([128, XC], bf16, tag="B2")
            B3 = pst.tile([128, XC], bf16, tag="B3")
            B4 = pst.tile([128, XC], bf16, tag="B4")

            ST2 = pst2.tile([28, STW], bf16, tag="ST2ALL")

            nc.vector.memset(X[:, 0:1], 0.0)
            # preload ACT function tables (Square/Sqrt) off the critical path
            nc.scalar.square(SQX[0:1, 0:2], ident[0:1, 0:2])
            nc.scalar.activation(M2[0:1, 0:2], ident[0:1, 0:2], Act.Sqrt, 0.0, 1.0)
            nc.scalar.activation(M2[0:1, 2:4], ident[0:1, 0:2], Act.Relu, 0.0, 1.0)

            # ---- load x: one staging tile [128, 896], col = 28*(s//128) +
            # 7b + c, partition = s % 128. One DMA per batch (strided dst,
            # 3-dim AP), then 8 PE transposes of [128, 112] (one per 512-seq
            # block T) -> psum [112, 128] (partition 28u + 7b + c), then 4
            # copies per T into X rows 32j+7b+c at col 24 + 512(T%2) + 128u.
            # One DMA per (batch, quarter of 8 seq-blocks): transposes for
            # quarter q start while quarter q+1 loads.
            stg = pstg.tile([128, 1024], f32, tag="stg")
            ndma = 0
            for qu in range(4):
                for b in range(BSH):
                    dst = dataclasses.replace(
                        stg[:],
                        ap=[[1024, 128], [32, 8], [1, 7]],
                        offset=256 * qu + 7 * b,
                    )
                    src = dataclasses.replace(
                        x_d.ap(),
                        ap=[[C, 128], [128 * C, 8], [1, C]],
                        offset=b * S * C + 1024 * qu * C,
                    )
                    eng = (nc.sync, nc.scalar, nc.gpsimd)[ndma % 3]
                    eng.dma_start(dst, src)
                    ndma += 1
            nblk = 0
            for T in range(8):
                j = T // 2
                pst_b = pps.tile([128, 2 * D], f32, tag="ps")
                pst_t = pst_b[:, 0:128]
                nc.tensor.transpose(
                    pst_t, stg[:, 128 * T : 128 * (T + 1)], ident[:]
                )
                for u in range(4):
                    c0 = 24 + 512 * (T % 2) + 128 * u
                    ceng = (nc.scalar.copy, nc.vector.tensor_copy)[nblk % 2]
                    ceng(
                        X[32 * j : 32 * j + 28, c0 : c0 + 128],
                        pst_b[32 * u : 32 * u + 28, 0:128],
                    )
                    nblk += 1
                if T % 2 == 1 and j + 1 < NJ:
                    # back-halo for chunk j+1: last 23 seq of chunk j
                    nc.vector.tensor_copy(
                        X[32 * (j + 1) : 32 * (j + 1) + 28, 1 : 1 + HALO],
                        pst_b[96 : 96 + 28, 128 - HALO : 128],
                    )
                if T == 0:
                    # j=0 front halo: replicate x[b,0,c] into cols 1..23
                    nc.vector.tensor_scalar(
                        X[0:28, 1 : 1 + HALO],
                        X[0:28, 24 : 24 + HALO],
                        0.0,
                        X[0:28, 24:25],
                        Alu.mult,
                        Alu.add,
                    )

            # ---- stats
            # window sums: P = cumsum(X), S24[m] = P[m] - P[m-24] (m >= 24;
            # col 0 of X is zero so the first window is exact).
            nc.scalar.square(SQX[:, 0:XC], X[:, 0:XC])
            nc.vector.tensor_tensor_scan(
                P[:, 0:XC], X[:, 0:XC], X[:, 0:XC], 0.0, Alu.add, Alu.bypass
            )
            nc.vector.tensor_tensor_scan(
                Q[:, 0:XC], SQX[:, 0:XC], SQX[:, 0:XC], 0.0, Alu.add, Alu.bypass
            )
            nc.vector.tensor_tensor(
                S24[:, 24:XC], P[:, 24:XC], P[:, 0:CH], Alu.subtract
            )
            nc.vector.tensor_tensor(
                SQ24[:, 24:XC], Q[:, 24:XC], Q[:, 0:CH], Alu.subtract
            )
            # var = max(SQ24 - S24^2/24, 0) / 23 ; std = sqrt(var)
            nc.scalar.activation(
                M2[:, 24:XC], S24[:, 24:XC], Act.Square, 0.0, float(W**-0.5)
            )
            nc.vector.tensor_tensor(
                SQ24[:, 24:XC], SQ24[:, 24:XC], M2[:, 24:XC], Alu.subtract
            )
            # clamp on ACT (Relu) to keep the DVE critical path short
            nc.scalar.activation(
                SQ24[:, 24:XC], SQ24[:, 24:XC], Act.Relu, 0.0, 1.0
            )

            # max/min ladders: w2/w4/w8/w16, then w24[m]=op(w16[m], w8[m-16])
            nc.scalar.copy(XB[:, 1:XC], X[:, 1:XC])
            nc.vector.tensor_tensor(
                B1[:, 2:XC], XB[:, 2:XC], XB[:, 1 : XC - 1], Alu.max
            )
            nc.vector.tensor_tensor(
                B3[:, 2:XC], XB[:, 2:XC], XB[:, 1 : XC - 1], Alu.min
            )
            nc.vector.tensor_tensor(
                B2[:, 4:XC], B1[:, 4:XC], B1[:, 2 : XC - 2], Alu.max
            )
            nc.vector.tensor_tensor(
                B4[:, 4:XC], B3[:, 4:XC], B3[:, 2 : XC - 2], Alu.min
            )
            nc.vector.tensor_tensor(
                B1[:, 8:XC], B2[:, 8:XC], B2[:, 4 : XC - 4], Alu.max
            )
            nc.vector.tensor_tensor(
                B3[:, 8:XC], B4[:, 8:XC], B4[:, 4 : XC - 4], Alu.min
            )
            nc.vector.tensor_tensor(
                B2[:, 16:XC], B1[:, 16:XC], B1[:, 8 : XC - 8], Alu.max
            )
            nc.vector.tensor_tensor(
                B4[:, 16:XC], B3[:, 16:XC], B3[:, 8 : XC - 8], Alu.min
            )

            # ---- finals write straight into ST2ALL (stat t at cols 4100t,
            # block col = 2 + seq). Order: x, mean(=S24), max, min, std.
            # F3 row 35k + 5c + t (tap k, channel c, stat t), row 105 = ones.
            # F3[r, m] = ST2[7b+c, 4100t + k + m]. Each tap is built in
            # chunk-aligned pieces so matmuls start after chunks 0-1:
            #   A: m in [2-k, 2050-k)   <- chunks 0,1
            #   B: m in [2050-k, 3074-k) <- chunk 2
            #   C: m in [3074-k, 4098)  <- chunk 3 + right wrap
            #   edge: m in [0, 2-k)     <- left wrap (k < 2)
            f3s = [
                pf3.tile([K, F3W], bf16, tag="F3", name=f"f3_{b}")
                for b in range(BSH)
            ]
            for b in range(BSH):
                nc.sync.dma_start(f3s[b][K - 1 : K, :], ones_d.ap())

            def sc(t, j):
                return SBW * t + 2 + CH * j

            def finals(j):
                r0, r1 = 32 * j, 32 * j + 28
                nc.scalar.copy(ST2[:, sc(0, j) : sc(0, j) + CH], X[r0:r1, 24:XC])
                nc.vector.tensor_tensor(
                    ST2[:, sc(1, j) : sc(1, j) + CH],
                    P[r0:r1, 24:XC],
                    P[r0:r1, 0:CH],
                    Alu.subtract,
                )
                nc.vector.tensor_tensor(
                    ST2[:, sc(2, j) : sc(2, j) + CH],
                    B2[r0:r1, 24:XC],
                    B1[r0:r1, 8 : 8 + CH],
                    Alu.max,
                )
                nc.vector.tensor_tensor(
                    ST2[:, sc(3, j) : sc(3, j) + CH],
                    B4[r0:r1, 24:XC],
                    B3[r0:r1, 8 : 8 + CH],
                    Alu.min,
                )
                nc.scalar.activation(
                    ST2[:, sc(4, j) : sc(4, j) + CH],
                    SQ24[r0:r1, 24:XC],
                    Act.Sqrt,
                    0.0,
                    1.0 / (W - 1),
                )

            def wrapcopy(dst_off, src_off):
                dst = dataclasses.replace(
                    ST2[:, 0:2], ap=[[STW, 28], [SBW, 5], [1, 2]], offset=dst_off
                )
                src = dataclasses.replace(
                    ST2[:, 0:2], ap=[[STW, 28], [SBW, 5], [1, 2]], offset=src_off
                )
                nc.vector.tensor_copy(dst, src)

            nf3 = 0

            def f3dma(b, k, m0, m1, eng=None):
                nonlocal nf3
                src = dataclasses.replace(
                    ST2[:],
                    ap=[[STW, 7], [SBW, 5], [1, m1 - m0]],
                    offset=7 * b * STW + k + m0,
                )
                if eng is None:
                    eng = (nc.sync, nc.scalar)[nf3 % 2]
                    nf3 += 1
                eng.dma_start(f3s[b][35 * k : 35 * k + 35, m0:m1], src)

            finals(0)
            finals(1)
            wrapcopy(2 + S, 2)  # right wrap <- seq 0:2 (chunk 0)
            for b in range(BSH):
                for k in range(3):
                    f3dma(b, k, 2 - k, 2050 - k)  # A
            finals(2)
            for b in range(BSH):
                for k in range(3):
                    f3dma(b, k, 2050 - k, 3074 - k)  # B
            finals(3)
            wrapcopy(0, 2 + S - 2)  # left wrap <- seq 4094:4096 (chunk 3)
            for b in range(BSH):
                for k in range(3):
                    f3dma(b, k, 3074 - k, F3W)  # C
                for k in range(2):
                    f3dma(b, k, 0, 2 - k, eng=nc.gpsimd)  # edge (left wraps)

            # ---- matmuls, group-major so early groups span all batches.
            # Group g < 3: tiles 1+8g .. 8+8g -> y rows 128+1024g ..;
            # last group: tiles 25..31 and 0 (needs the edge cols).
            GROUPS = [list(range(1 + 8 * g, 9 + 8 * g)) for g in range(3)]
            GROUPS.append([25, 26, 27, 28, 29, 30, 31, 0])
            ncopy = 0
            for gi, tiles in enumerate(GROUPS):
                for b in range(BSH):
                    stage = pout.tile([128, OUTG * D], bf16, tag="stage")
                    for pi in range(4):
                        ps = pps.tile([128, 2 * D], f32, tag="ps")
                        for h in range(2):
                            q = tiles[2 * pi + h]
                            nc.tensor.matmul(
                                ps[:, D * h : D * (h + 1)],
                                f3s[b][:, 128 * q + 1 : 128 * q + 129],
                                wt[:],
                                start=True,
                                stop=True,
                            )
                        ceng = (
                            nc.vector.tensor_copy
                            if ncopy % 8 in (1, 4, 6)
                            else nc.scalar.copy
                        )
                        ceng(stage[:, 1024 * pi : 1024 * (pi + 1)], ps[:])
                        ncopy += 1
                    deng = nc.sync if (gi * 4 + b) % 2 == 0 else nc.scalar
                    if gi < 3:
                        deng.dma_start(
                            y_d.ap()[
                                b, 128 + 1024 * gi : 128 + 1024 * (gi + 1), :
                            ].rearrange("(q p) d -> p q d", p=128),
                            stage[:].rearrange("p (q d) -> p q d", q=OUTG),
                        )
                    else:
                        deng.dma_start(
                            y_d.ap()[b, 3200:S, :].rearrange(
                                "(q p) d -> p q d", p=128
                            ),
                            stage[:, 0 : 7 * D].rearrange(
                                "p (q d) -> p q d", q=7
                            ),
                        )
                        deng.dma_start(
                            y_d.ap()[b, 0:128, :], stage[:, 7 * D : 8 * D]
                        )

    nc.compile()
    return nc


def _prep_host(W_conv, b_conv):
    import ml_dtypes

    # wt row = 35k + 5c + t; conv in-channel = 7t + c; mean fold 1/24.
    wkf = np.ascontiguousarray(W_conv.transpose(2, 1, 0)).copy()  # (3, 35, 512)
    wkf[:, C : 2 * C, :] *= 1.0 / W
    wt = np.empty((K, D), np.float32)
    kk, tt, cc = np.meshgrid(
        np.arange(3), np.arange(5), np.arange(C), indexing="ij"
    )
    wt[(35 * kk + 5 * cc + tt).ravel()] = wkf.reshape(3, 35, D)[
        kk.ravel(), (7 * tt + cc).ravel()
    ]
    wt[K - 1] = b_conv.astype(np.float32)
    return wt.astype(ml_dtypes.bfloat16)


def _run(x, W_conv, b_conv, trace=False, **kw):
    from concourse import bass_utils

    if "nc" not in _CACHE:
        _CACHE["nc"] = _build()
    nc = _CACHE["nc"]

    wt = _prep_host(np.asarray(W_conv), np.asarray(b_conv))
    import ml_dtypes

    ones = np.ones((1, F3W), ml_dtypes.bfloat16)
    ident = np.eye(128, dtype=np.float32)
    x = np.ascontiguousarray(np.asarray(x, np.float32))
    in_maps = [
        {"x": x[BSH * i : BSH * (i + 1)], "wt": wt, "ones": ones, "ident": ident}
        for i in range(NCORES)
    ]
    res = bass_utils.run_bass_kernel_spmd(
        nc, in_maps, core_ids=list(range(NCORES)), trace=trace, **kw
    )
    out = np.concatenate(
        [np.asarray(r["y"]).astype(np.float32) for r in res.results], axis=0
    )
    return out, res


def kernel(x, x_mark=None, W_conv=None, b_conv=None, **_unused):
    out, _ = _run(x, W_conv, b_conv, trace=False)
    return out


# revision 16
# speedup vs baseline: 1.2116x; 1.0582x over previous
"""Trainium2 Bass kernel for nn_DataEmbedding_Stats.

Computation: rolling-window stats (window=24, replicate-padded) over
x (B,S,7) -> 35 features -> circular conv1d(k=3) -> (B,S,512).

Strategy (8 NeuronCores, pure data parallel over batch, 4 batches/core):
 - x loaded as 32 [128,28] staging DMAs + PE transposes into
   X [128, 1048]: partition = 32j + 7b + c (j = 1024-seq chunk), col 0 = 0,
   cols 1..23 = replicate halo, col 24+s = x at in-chunk seq s.
 - window sums via cumsum (tensor_tensor_scan) + shift-24 subtract; var from
   sum/sumsq; max/min via log-doubling ladders (levels 2+ in bf16).
 - stats finals write bf16 directly into ST2ALL [28, 5*4100]: partition
   7b+c, stat t block at cols 4100t, block col 2+s (+2-col wraps for the
   circular conv).
 - F3 [106, 4098] per batch (im2col for the k=3 conv) built with 3 DMAs
   (one per tap k): dst rows 35k + 5c + t, src = ST2ALL[7b+c, 4100t+k+m]
   via a custom 3-dim access pattern. Weight rows reordered on host to
   match; row 105 = ones (bias folded); mean = S24 * (W/24 fold).
 - conv as matmul: per 128 positions, psum[128,512] = F3[:,p+1:p+129].T @ wt.
 - PSUM -> SBUF copies downcast to bf16 (DVE/ACT alternating); bf16 output
   staged [128, 4096] and stored as 1 MB DMAs on both HWDGE queues; host
   upcasts to f32.
"""

import dataclasses

import numpy as np

try:
    import concourse.bass as bass  # noqa: F401
except ImportError:
    import sys

    for _p in ("/opt/trn_rl_repo", "/root/.axon_site/_ro/trn_rl_repo"):
        if _p not in sys.path:
            sys.path.insert(0, _p)

B, S, C, W, D = 32, 4096, 7, 24, 512
NCORES = 8
BSH = B // NCORES          # batches per core
NJ = 4                     # seq chunks (row groups of 32 partitions)
CH = S // NJ               # 1024
HALO = W - 1               # 23
XC = 1 + HALO + CH         # 1048: col 0 = 0, 1..23 halo, 24+s data
NF = 5 * C                 # 35 features
K = 3 * NF + 1             # 106 contraction rows (ones row last)
F3W = S + 2                # 4098
STW = 5 * CH               # 5120: ST2Q width (5 per-chunk stat blocks of 1024)
NT = S // 128              # 32 position tiles per batch
OUTG = 8                   # position tiles per output staging tile

_CACHE = {}


def _build():
    import concourse.bacc as bacc
    import concourse.tile as tile
    from concourse import mybir

    f32 = mybir.dt.float32
    bf16 = mybir.dt.bfloat16
    Alu = mybir.AluOpType
    Act = mybir.ActivationFunctionType

    nc = bacc.Bacc(
        "TRN2",
        target_bir_lowering=False,
        debug=False,
        enable_asserts=False,
        num_devices=NCORES,
    )

    x_d = nc.dram_tensor("x", (BSH, S, C), f32, kind="ExternalInput")
    wt_d = nc.dram_tensor("wt", (K, D), bf16, kind="ExternalInput")
    ones_d = nc.dram_tensor("ones", (1, F3W), bf16, kind="ExternalInput")
    id_d = nc.dram_tensor("ident", (128, 128), f32, kind="ExternalInput")
    y_d = nc.dram_tensor("y", (BSH, S, D), bf16, kind="ExternalOutput")

    with tile.TileContext(nc) as tc:
        with (
            tc.tile_pool(name="stats", bufs=1) as pst,
            tc.tile_pool(name="st2p", bufs=1) as pst2,
            tc.tile_pool(name="f3p", bufs=4) as pf3,
            tc.tile_pool(name="wtp", bufs=1) as pwt,
            tc.tile_pool(name="stage_in", bufs=12) as pstg,
            tc.tile_pool(name="psum", bufs=4, space="PSUM") as pps,
            tc.tile_pool(name="outp", bufs=4) as pout,
        ):
            wt = pwt.tile([K, D], bf16, tag="wt")
            nc.sync.dma_start(wt[:], wt_d.ap())
            ident = pwt.tile([128, 128], f32, tag="ident")
            nc.sync.dma_start(ident[:], id_d.ap())

            X = pst.tile([128, XC], f32, tag="X")
            SQX = pst.tile([128, XC], f32, tag="SQX")
            P = pst.tile([128, XC], f32, tag="P")
            Q = pst.tile([128, XC], f32, tag="Q")
            S24 = pst.tile([128, XC], f32, tag="S24")
            SQ24 = pst.tile([128, XC], f32, tag="SQ24")
            M2 = pst.tile([128, XC], f32, tag="M2")
            XB = pst.tile([128, XC], bf16, tag="XB")
            B1 = pst.tile([128, XC], bf16, tag="B1")
            B2 = pst.tile([128, XC], bf16, tag="B2")
            B3 = pst.tile([128, XC], bf16, tag="B3")
            B4 = pst.tile([128, XC], bf16, tag="B4")

            ST2 = pst2.tile([128, STW], bf16, tag="ST2ALL")

            nc.vector.memset(X[:, 0:1], 0.0)
            # preload ACT function tables (Square/Sqrt) off the critical path
            nc.scalar.square(SQX[0:1, 0:2], ident[0:1, 0:2])
            nc.scalar.activation(M2[0:1, 0:2], ident[0:1, 0:2], Act.Sqrt, 0.0, 1.0)
            nc.scalar.activation(M2[0:1, 2:4], ident[0:1, 0:2], Act.Relu, 0.0, 1.0)

            # ---- load x: one staging tile [128, 896], col = 28*(s//128) +
            # 7b + c, partition = s % 128. One DMA per batch (strided dst,
            # 3-dim AP), then 8 PE transposes of [128, 112] (one per 512-seq
            # block T) -> psum [112, 128] (partition 28u + 7b + c), then 4
            # copies per T into X rows 32j+7b+c at col 24 + 512(T%2) + 128u.
            # One DMA per (batch, quarter of 8 seq-blocks): transposes for
            # quarter q start while quarter q+1 loads.
            stg = pstg.tile([128, 1024], f32, tag="stg")
            ndma = 0
            for qu in range(4):
                for b in range(BSH):
                    dst = dataclasses.replace(
                        stg[:],
                        ap=[[1024, 128], [32, 8], [1, 7]],
                        offset=256 * qu + 7 * b,
                    )
                    src = dataclasses.replace(
                        x_d.ap(),
                        ap=[[C, 128], [128 * C, 8], [1, C]],
                        offset=b * S * C + 1024 * qu * C,
                    )
                    eng = (nc.sync, nc.scalar, nc.gpsimd)[ndma % 3]
                    eng.dma_start(dst, src)
                    ndma += 1
            nblk = 0
            for T in range(8):
                j = T // 2
                pst_b = pps.tile([128, 2 * D], f32, tag="ps")
                pst_t = pst_b[:, 0:128]
                nc.tensor.transpose(
                    pst_t, stg[:, 128 * T : 128 * (T + 1)], ident[:]
                )
                for u in range(4):
                    c0 = 24 + 512 * (T % 2) + 128 * u
                    ceng = (nc.scalar.copy, nc.vector.tensor_copy)[nblk % 2]
                    ceng(
                        X[32 * j : 32 * j + 28, c0 : c0 + 128],
                        pst_b[32 * u : 32 * u + 28, 0:128],
                    )
                    nblk += 1
                if T % 2 == 1 and j + 1 < NJ:
                    # back-halo for chunk j+1: last 23 seq of chunk j
                    nc.vector.tensor_copy(
                        X[32 * (j + 1) : 32 * (j + 1) + 28, 1 : 1 + HALO],
                        pst_b[96 : 96 + 28, 128 - HALO : 128],
                    )
                if T == 0:
                    # j=0 front halo: replicate x[b,0,c] into cols 1..23
                    nc.vector.tensor_scalar(
                        X[0:28, 1 : 1 + HALO],
                        X[0:28, 24 : 24 + HALO],
                        0.0,
                        X[0:28, 24:25],
                        Alu.mult,
                        Alu.add,
                    )

            # ---- stats
            # window sums: P = cumsum(X), S24[m] = P[m] - P[m-24] (m >= 24;
            # col 0 of X is zero so the first window is exact).
            nc.scalar.square(SQX[:, 0:XC], X[:, 0:XC])
            nc.vector.tensor_tensor_scan(
                P[:, 0:XC], X[:, 0:XC], X[:, 0:XC], 0.0, Alu.add, Alu.bypass
            )
            nc.vector.tensor_tensor_scan(
                Q[:, 0:XC], SQX[:, 0:XC], SQX[:, 0:XC], 0.0, Alu.add, Alu.bypass
            )
            nc.vector.tensor_tensor(
                S24[:, 24:XC], P[:, 24:XC], P[:, 0:CH], Alu.subtract
            )
            nc.vector.tensor_tensor(
                SQ24[:, 24:XC], Q[:, 24:XC], Q[:, 0:CH], Alu.subtract
            )
            # var = max(SQ24 - S24^2/24, 0) / 23 ; std = sqrt(var)
            nc.scalar.activation(
                M2[:, 24:XC], S24[:, 24:XC], Act.Square, 0.0, float(W**-0.5)
            )
            nc.vector.tensor_tensor(
                SQ24[:, 24:XC], SQ24[:, 24:XC], M2[:, 24:XC], Alu.subtract
            )
            # clamp on ACT (Relu) to keep the DVE critical path short
            nc.scalar.activation(
                SQ24[:, 24:XC], SQ24[:, 24:XC], Act.Relu, 0.0, 1.0
            )

            # max/min ladders: w2/w4/w8/w16, then w24[m]=op(w16[m], w8[m-16])
            nc.scalar.copy(XB[:, 1:XC], X[:, 1:XC])
            nc.vector.tensor_tensor(
                B1[:, 2:XC], XB[:, 2:XC], XB[:, 1 : XC - 1], Alu.max
            )
            nc.vector.tensor_tensor(
                B3[:, 2:XC], XB[:, 2:XC], XB[:, 1 : XC - 1], Alu.min
            )
            nc.vector.tensor_tensor(
                B2[:, 4:XC], B1[:, 4:XC], B1[:, 2 : XC - 2], Alu.max
            )
            nc.vector.tensor_tensor(
                B4[:, 4:XC], B3[:, 4:XC], B3[:, 2 : XC - 2], Alu.min
            )
            nc.vector.tensor_tensor(
                B1[:, 8:XC], B2[:, 8:XC], B2[:, 4 : XC - 4], Alu.max
            )
            nc.vector.tensor_tensor(
                B3[:, 8:XC], B4[:, 8:XC], B4[:, 4 : XC - 4], Alu.min
            )
            nc.vector.tensor_tensor(
                B2[:, 16:XC], B1[:, 16:XC], B1[:, 8 : XC - 8], Alu.max
            )
            nc.vector.tensor_tensor(
                B4[:, 16:XC], B3[:, 16:XC], B3[:, 8 : XC - 8], Alu.min
            )

            # ---- finals write straight into ST2ALL (stat t at cols 4100t,
            # block col = 2 + seq). Order: x, mean(=S24), max, min, std.
            # F3 row 35k + 5c + t (tap k, channel c, stat t), row 105 = ones.
            # F3[r, m] = ST2[7b+c, 4100t + k + m]. Each tap is built in
            # chunk-aligned pieces so matmuls start after chunks 0-1:
            #   A: m in [2-k, 2050-k)   <- chunks 0,1
            #   B: m in [2050-k, 3074-k) <- chunk 2
            #   C: m in [3074-k, 4098)  <- chunk 3 + right wrap
            #   edge: m in [0, 2-k)     <- left wrap (k < 2)
            f3s = [
                pf3.tile([K, F3W], bf16, tag="F3", name=f"f3_{b}")
                for b in range(BSH)
            ]
            for b in range(BSH):
                nc.sync.dma_start(f3s[b][K - 1 : K, :], ones_d.ap())

            def finals(j):
                r0, r1 = 32 * j, 32 * j + 28

                def dst(t):
                    return ST2[r0:r1, CH * t : CH * (t + 1)]

                nc.scalar.copy(dst(0), X[r0:r1, 24:XC])
                nc.vector.tensor_tensor(
                    dst(1), P[r0:r1, 24:XC], P[r0:r1, 0:CH], Alu.subtract
                )
                nc.vector.tensor_tensor(
                    dst(2), B2[r0:r1, 24:XC], B1[r0:r1, 8 : 8 + CH], Alu.max
                )
                nc.vector.tensor_tensor(
                    dst(3), B4[r0:r1, 24:XC], B3[r0:r1, 8 : 8 + CH], Alu.min
                )
                nc.scalar.activation(
                    dst(4), SQ24[r0:r1, 24:XC], Act.Sqrt, 0.0, 1.0 / (W - 1)
                )

            for j in range(NJ):
                finals(j)

            # F3 main: per (b, tap k, chunk j) one DMA [7c, 5t, 1024m] with a
            # fully contiguous per-partition source run; wrap edges as tiny
            # DMAs from chunk 3 / chunk 0.
            nf3 = 0

            def f3src(part0, col0, n):
                return dataclasses.replace(
                    ST2[:],
                    ap=[[STW, 7], [CH, 5], [1, n]],
                    offset=part0 * STW + col0,
                )

            def build_f3(b):
                nonlocal nf3
                for k in range(3):
                    for j in range(NJ):
                        m0 = CH * j + 2 - k
                        m1 = min(CH * (j + 1) + 2 - k, F3W)
                        eng = (nc.sync, nc.scalar)[nf3 % 2]
                        nf3 += 1
                        eng.dma_start(
                            f3s[b][35 * k : 35 * k + 35, m0:m1],
                            f3src(32 * j + 7 * b, 0, m1 - m0),
                        )
                # edges: left (k=0 cols 0:2, k=1 col 0) <- seq 4094+, right
                # (k=1 col 4097, k=2 cols 4096:4098) <- seq 0+
                nc.gpsimd.dma_start(
                    f3s[b][0:35, 0:2], f3src(96 + 7 * b, CH - 2, 2)
                )
                nc.gpsimd.dma_start(
                    f3s[b][35:70, 0:1], f3src(96 + 7 * b, CH - 1, 1)
                )
                nc.gpsimd.dma_start(
                    f3s[b][35:70, F3W - 1 : F3W], f3src(7 * b, 0, 1)
                )
                nc.gpsimd.dma_start(
                    f3s[b][70:105, F3W - 2 : F3W], f3src(7 * b, 0, 2)
                )

            for b in range(BSH):
                build_f3(b)

            # ---- matmuls, batch-major (f3s[b] fully built before use)
            ncopy = 0
            for b in range(BSH):
                for gi in range(NT // OUTG):
                    stage = pout.tile([128, OUTG * D], bf16, tag="stage")
                    for pi in range(4):
                        ps = pps.tile([128, 2 * D], f32, tag="ps")
                        for h in range(2):
                            q = OUTG * gi + 2 * pi + h
                            nc.tensor.matmul(
                                ps[:, D * h : D * (h + 1)],
                                f3s[b][:, 128 * q + 1 : 128 * q + 129],
                                wt[:],
                                start=True,
                                stop=True,
                            )
                        ceng = (
                            nc.vector.tensor_copy
                            if ncopy % 8 in (1, 4, 6)
                            else nc.scalar.copy
                        )
                        ceng(stage[:, 1024 * pi : 1024 * (pi + 1)], ps[:])
                        ncopy += 1
                    deng = nc.sync if (b * 4 + gi) % 2 == 0 else nc.scalar
                    deng.dma_start(
                        y_d.ap()[
                            b, 1024 * gi : 1024 * (gi + 1), :
                        ].rearrange("(q p) d -> p q d", p=128),
                        stage[:].rearrange("p (q d) -> p q d", q=OUTG),
                    )

    nc.compile()
    return nc


def _prep_host(W_conv, b_conv):
    import ml_dtypes

    # wt row = 35k + 5c + t; conv in-channel = 7t + c; mean fold 1/24.
    wkf = np.ascontiguousarray(W_conv.transpose(2, 1, 0)).copy()  # (3, 35, 512)
    wkf[:, C : 2 * C, :] *= 1.0 / W
    wt = np.empty((K, D), np.float32)
    kk, tt, cc = np.meshgrid(
        np.arange(3), np.arange(5), np.arange(C), indexing="ij"
    )
    wt[(35 * kk + 5 * cc + tt).ravel()] = wkf.reshape(3, 35, D)[
        kk.ravel(), (7 * tt + cc).ravel()
    ]
    wt[K - 1] = b_conv.astype(np.float32)
    return wt.astype(ml_dtypes.bfloat16)


def _run(x, W_conv, b_conv, trace=False, **kw):
    from concourse import bass_utils

    if "nc" not in _CACHE:
        _CACHE["nc"] = _build()
    nc = _CACHE["nc"]

    wt = _prep_host(np.asarray(W_conv), np.asarray(b_conv))
    import ml_dtypes

    ones = np.ones((1, F3W), ml_dtypes.bfloat16)
    ident = np.eye(128, dtype=np.float32)
    x = np.ascontiguousarray(np.asarray(x, np.float32))
    in_maps = [
        {"x": x[BSH * i : BSH * (i + 1)], "wt": wt, "ones": ones, "ident": ident}
        for i in range(NCORES)
    ]
    res = bass_utils.run_bass_kernel_spmd(
        nc, in_maps, core_ids=list(range(NCORES)), trace=trace, **kw
    )
    out = np.concatenate(
        [np.asarray(r["y"]).astype(np.float32) for r in res.results], axis=0
    )
    return out, res


def kernel(x, x_mark=None, W_conv=None, b_conv=None, **_unused):
    out, _ = _run(x, W_conv, b_conv, trace=False)
    return out
